# revision 8
# baseline (speedup 1.0000x reference)
"""Multi-head attention (B=4, S=2048, D=512, H=8) on 8 TRN2 NeuronCores.

Sharding: core c handles batch b = c//2 and head-group g = c%2 (heads
4g..4g+3, a 256-dim slice of the model). Attention is independent per
(batch, head); the out-projection contracts over all 512 dims, so each core
computes a partial out = attn_g @ W_out[g] and the host sums the two
partials per batch. x^T inputs are marshalled on the host; projection
weights are sliced per head-group (no K/V duplication).

Per-core dataflow:
  1. qT/kT = (x W)^T in [256, 2048] dims-major fp32r layout; v in natural
     [2048, 256] + a ones column per (key-block, head) -> v_aug fp16.
  2. Per slot (query-half, head): scores^T st[128k, 1024q] fp32r in PSUM;
     softmax numerators pT[128k, 1024q] fp16 produced by a mixed engine
     schedule: ACT exact exp / DVE+Pool Schraudolph int16-bitcast exp
     (single or phase-averaged pair for 4x better accuracy). Pool cannot
     read PSUM, so its inputs are DMA-staged to SBUF.
  3. PV in natural orientation: pv[128q, 65] += pT^T @ v_aug per
     (q-block, key-block) - the 65-wide free dim (64 dims + denominator
     column) makes each accumulation step cost 65 PE cycles instead of
     the 1024 a dims-major PV would pay. Normalization (1/denominator)
     is fused into the PSUM->SBUF copy via a per-partition scalar.
  4. attn[q,dh] tiles are transposed back to dims-major via PE-transpose
     against a host-provided identity, then out = attnT^T @ W_out (fp16)
     streams per 128-query chunk.
"""

import numpy as np

import concourse.bass as bass
from concourse import bacc
import concourse.mybir as mybir
import concourse.tile as tile
from concourse.bass_utils import run_bass_kernel_spmd

B, S, D, H = 4, 2048, 512, 8
DH = 64
P = 128
NCORES = 8
HG = H // 2          # 4 heads per core
DG = HG * DH         # 256 dims per core
NKB = S // P         # 16 key blocks
VW = DH + 1          # 65
F32 = mybir.dt.float32
F32R = mybir.dt.float32r
F16 = mybir.dt.float16
I16 = mybir.dt.int16
EXP = mybir.ActivationFunctionType.Exp
MUL = mybir.AluOpType.mult
ADD = mybir.AluOpType.add
SCALE = 1.0 / np.sqrt(DH)            # 0.125
LOG2E = 1.4426950408889634
A16 = SCALE * LOG2E * 1024.0         # fp16-bits Schraudolph slope
B_SS = 15.0 * 1024.0 - 59.0          # single Schraudolph bias (mean-1)
# Equal-weight phase pair: exp(x) ~ bits(x*A+B_PH1).f16 + bits(x*A+B_PH2).f16
# (CV 0.53%, zero mean bias; the 0.5 weights are folded into the biases so
# the combine is a single fp16 add, which Pool supports.)
B_PH1 = 15.0 * 1024.0 - 1024.0 - 332.0
B_PH2 = B_PH1 + 503.5

# Per-slot exp engine schedule over the 16 key blocks:
#  A  = exact exp on ACT
#  D  = single Schraudolph on DVE (from PSUM)
#  PH = phase-averaged pair: two DVE Schraudolphs + Pool combine (SBUF fp16)
# (Pool/GPSIMD cannot read PSUM, and neither can DMA, so Pool only gets the
# all-SBUF combine step.)
SCHED = ["A", "PH", "A", "D", "A", "A", "PH", "A",
         "A", "D", "A", "A", "PH", "A", "A", "A"]


def _build_mha(tc, out_d, xqT_d, xkT_d, xvT_d, wq_d, wk_d, wv_d, wo_d, ident_d):
    nc = tc.nc

    dma_rr = [0]

    def dma(dst, src):
        eng = nc.sync if dma_rr[0] % 2 == 0 else nc.scalar
        dma_rr[0] += 1
        eng.dma_start(dst, src)

    copy_rr = [0]

    def pcopy(dst, src):
        if copy_rr[0] % 2 == 0:
            nc.scalar.copy(dst, src)
        else:
            nc.vector.tensor_copy(dst, src)
        copy_rr[0] += 1

    with (
        tc.tile_pool(name="consts", bufs=1) as cpool,
        tc.tile_pool(name="big", bufs=1) as bpool,
        tc.tile_pool(name="work", bufs=2) as wpool,
    ):
        # x chunk loaders go out first so projections can start early.
        def load_x(xT_d, c, col, issuer=None):
            t = wpool.tile([P, 512], F32R, tag="xT", bufs=10)
            src = xT_d[c * P : (c + 1) * P, col * 512 : (col + 1) * 512].bitcast(F32R)
            if issuer is None:
                dma(t, src)
            else:
                issuer.dma_start(t, src)
            return t

        first_xq = [load_x(xqT_d, c, 0) for c in range(4)]

        # Weights + identity (SP queue, overlapped with the x stream above).
        wq_sb = cpool.tile([P, 4, DG], F32R)
        wk_sb = cpool.tile([P, 4, DG], F32R)
        wv_sb = cpool.tile([P, 4, DG], F32R)
        wo_sb = cpool.tile([P, 2, D], F16)
        ident = cpool.tile([P, P], F16)
        nc.sync.dma_start(ident, ident_d)
        for w_sb, w_d in ((wq_sb, wq_d), (wk_sb, wk_d), (wv_sb, wv_d)):
            wr = w_d.rearrange("(c p) n -> p c n", p=P).bitcast(F32R)
            for c in range(4):
                nc.sync.dma_start(w_sb[:, c, :], wr[:, c, :])
        wor = wo_d.rearrange("(c p) n -> p c n", p=P)
        for c in range(2):
            nc.sync.dma_start(wo_sb[:, c, :], wor[:, c, :])

        qT = bpool.tile([P, 2, S], F32R)     # [dim%128, dim//128, q]
        kT = bpool.tile([P, 2, S], F32R)
        v_aug = bpool.tile([P, NKB, HG, VW], F16)  # [k%128, k//128, h, dh|1]
        attnT = bpool.tile([P, 2, S], F16)   # [dim%128, dim//128, q]

        # Pull the ACT exp-table load to t=0.
        warm = cpool.tile([P, 1], F16)
        nc.scalar.activation(warm, ident[:, 0:1], EXP)

        # Ones column per (key-block, head) for the fused denominator.
        nc.gpsimd.tensor_scalar(
            out=v_aug.rearrange("p a b e -> p (a b) e")[:, :, DH],
            in0=ident[:, 0:1].broadcast_to([P, NKB * HG]),
            scalar1=0.0,
            scalar2=1.0,
            op0=MUL,
            op1=ADD,
        )

        # ---------------- phase A: projections ----------------
        with tc.tile_pool(name="psA", bufs=4, space="PSUM") as psA:
            def project_T(xT_d, w_sb, dst, preloaded=None):
                for col in range(4):
                    if col == 0 and preloaded is not None:
                        xts = preloaded
                    else:
                        xts = [load_x(xT_d, c, col) for c in range(4)]
                    for mc in range(2):
                        pp = psA.tile([P, 512], F32, tag="qk")
                        for c in range(4):
                            nc.tensor.matmul(
                                pp,
                                w_sb[:, c, mc * P : (mc + 1) * P],
                                xts[c],
                                start=(c == 0),
                                stop=(c == 3),
                            )
                        pcopy(dst[:, mc, col * 512 : (col + 1) * 512], pp)

            project_T(xqT_d, wq_sb, qT, preloaded=first_xq)
            project_T(xkT_d, wk_sb, kT)

            # V projection: natural [keys, 256] scattered into v_aug.
            for col in range(4):
                xvs = [load_x(xvT_d, c, col) for c in range(4)]
                for kb in range(4):
                    kblk = col * 4 + kb
                    pp = psA.tile([P, DG], F32, tag="v", bufs=2)
                    for c in range(4):
                        nc.tensor.matmul(
                            pp,
                            xvs[c][:, kb * P : (kb + 1) * P],
                            wv_sb[:, c, :],
                            start=(c == 0),
                            stop=(c == 3),
                        )
                    pcopy(
                        v_aug[:, kblk, :, 0:DH],
                        pp.rearrange("p (h e) -> p h e", e=DH),
                    )

        # ---------------- attention slots ----------------
        slots = [(qh, h) for qh in (0, 1) for h in range(4)]
        st_cm = tc.tile_pool(name="ps_st", bufs=2, space="PSUM")
        pv_cm = tc.tile_pool(name="ps_pv", bufs=1, space="PSUM")
        aT_cm = tc.tile_pool(name="ps_aT", bufs=1, space="PSUM")
        opj_cm = tc.tile_pool(name="ps_opj", bufs=1, space="PSUM")
        st_pool = st_cm.__enter__()
        pv_pool = pv_cm.__enter__()
        aT_pool = aT_cm.__enter__()
        opj_pool = opj_cm.__enter__()

        pT_tiles = {}
        attn_tiles = {}
        combines = []  # deferred PH combine emissions

        def emit_score_block(i, kblk):
            qh, h = slots[i]
            po, mc = DH * (h % 2), h // 2
            pT = pT_tiles[i]
            st = st_pool.tile([P, 1024], F32, tag="st")
            qoff = qh * 1024
            for qc in range(2):
                nc.tensor.matmul(
                    st[:, qc * 512 : (qc + 1) * 512],
                    kT[po : po + DH, mc, kblk * P : (kblk + 1) * P],
                    qT[po : po + DH, mc, qoff + qc * 512 : qoff + (qc + 1) * 512],
                    start=True,
                    stop=True,
                )
            kind = SCHED[kblk]
            dst16 = pT[:, kblk, :]
            if kind == "A":
                nc.scalar.activation(dst16, st, EXP, scale=float(SCALE))
            elif kind == "D":
                nc.vector.tensor_scalar(
                    out=dst16.bitcast(I16), in0=st,
                    scalar1=A16, scalar2=B_SS, op0=MUL, op1=ADD,
                )
            else:  # PH
                s1 = wpool.tile([P, 1024], I16, tag="ph1", bufs=3)
                nc.vector.tensor_scalar(
                    out=s1, in0=st, scalar1=A16, scalar2=B_PH1, op0=MUL, op1=ADD,
                )
                s2 = wpool.tile([P, 1024], I16, tag="ph2", bufs=3)
                nc.vector.tensor_scalar(
                    out=s2, in0=st,
                    scalar1=A16, scalar2=B_PH2, op0=MUL, op1=ADD,
                )
                combines.append((dst16, s1, s2))

        def emit_combines():
            while combines:
                dst16, s1, s2 = combines.pop(0)
                nc.gpsimd.tensor_tensor(
                    out=dst16, in0=s1.bitcast(F16), in1=s2.bitcast(F16), op=ADD
                )

        def emit_pv_half(j, half):
            qh, h = slots[j]
            pT = pT_tiles[j]
            pv = pv_pool.tile([P, 4, VW], F32, tag="pv")
            for jj in range(4):
                qblk = half * 4 + jj
                qc0 = qh * 1024 + qblk * P
                for kblk in range(NKB):
                    nc.tensor.matmul(
                        pv[:, jj, :],
                        pT[:, kblk, qblk * P : (qblk + 1) * P],
                        v_aug[:, kblk, h, :],
                        start=(kblk == 0),
                        stop=(kblk == NKB - 1),
                    )
            return pv

        def emit_norm_half(j, half, pv):
            if half == 0:
                attn_tiles[j] = wpool.tile([P, 8, DH], F16, tag="attn", bufs=2, name=f"attn{j}")
            attn = attn_tiles[j]
            recip = wpool.tile([P, 4], F32, tag="recip", bufs=2)
            nc.vector.reciprocal(recip, pv[:, :, DH])
            for jj in range(4):
                nc.vector.tensor_scalar(
                    out=attn[:, half * 4 + jj, :],
                    in0=pv[:, jj, 0:DH],
                    scalar1=recip[:, jj : jj + 1],
                    scalar2=None,
                    op0=MUL,
                )

        def emit_transpose(j):
            qh, h = slots[j]
            po, mc = DH * (h % 2), h // 2
            attn = attn_tiles.pop(j)
            aT = aT_pool.tile([DH, 1024], F16, tag="aT")
            for qblk in range(8):
                nc.tensor.transpose(
                    aT[:, qblk * P : (qblk + 1) * P], attn[:, qblk, :], ident
                )
            nc.vector.tensor_copy(
                attnT[po : po + DH, mc, qh * 1024 : (qh + 1) * 1024], aT
            )

        def emit_opj(qch):
            op = opj_pool.tile([P, D], F32, tag="opj")
            for c in range(2):
                nc.tensor.matmul(
                    op,
                    attnT[:, c, qch * P : (qch + 1) * P],
                    wo_sb[:, c, :],
                    start=(c == 0),
                    stop=(c == 1),
                )
            ob = wpool.tile([P, D], F32, tag="ob", bufs=4)
            nc.vector.tensor_copy(ob, op)
            nc.sync.dma_start(out_d[qch * P : (qch + 1) * P, :], ob)

        # opj chunk schedule: virtual slot -> list of query chunks
        opj_sched = {6: [0, 1, 2, 3], 7: [4, 5, 6, 7], 9: [8, 9, 10, 11, 12, 13, 14, 15]}

        for i in range(10):
            pv_prev = None
            if i <= 7:
                pT_tiles[i] = wpool.tile([P, NKB, 1024], F16, tag="pT", bufs=2, name=f"pT{i}")
                for kblk in range(8):
                    emit_score_block(i, kblk)
                if i - 1 >= 0:
                    pv_prev = emit_pv_half(i - 1, 0)
                for kblk in range(8, NKB):
                    emit_score_block(i, kblk)
                if i - 2 >= 0:
                    emit_transpose(i - 2)
                if pv_prev is not None:
                    emit_norm_half(i - 1, 0, pv_prev)
                    pv_prev2 = emit_pv_half(i - 1, 1)
                    emit_norm_half(i - 1, 1, pv_prev2)
                    del pT_tiles[i - 1]
                emit_combines()
            else:
                if i - 1 <= 7:
                    pv_prev = emit_pv_half(i - 1, 0)
                    emit_norm_half(i - 1, 0, pv_prev)
                    pv_prev2 = emit_pv_half(i - 1, 1)
                    emit_norm_half(i - 1, 1, pv_prev2)
                    del pT_tiles[i - 1]
                if i - 2 <= 7:
                    emit_transpose(i - 2)
            for qch in opj_sched.get(i, []):
                emit_opj(qch)

        opj_cm.__exit__(None, None, None)
        aT_cm.__exit__(None, None, None)
        pv_cm.__exit__(None, None, None)
        st_cm.__exit__(None, None, None)


_CACHED_NC = None


def _get_nc():
    global _CACHED_NC
    if _CACHED_NC is not None:
        return _CACHED_NC
    nc = bacc.Bacc("TRN2", target_bir_lowering=False, debug=False)
    xqT = nc.dram_tensor("xqT", [D, S], F32, kind="ExternalInput").ap()
    xkT = nc.dram_tensor("xkT", [D, S], F32, kind="ExternalInput").ap()
    xvT = nc.dram_tensor("xvT", [D, S], F32, kind="ExternalInput").ap()
    wq = nc.dram_tensor("wq", [D, DG], F32, kind="ExternalInput").ap()
    wk = nc.dram_tensor("wk", [D, DG], F32, kind="ExternalInput").ap()
    wv = nc.dram_tensor("wv", [D, DG], F32, kind="ExternalInput").ap()
    wo = nc.dram_tensor("wo", [DG, D], F16, kind="ExternalInput").ap()
    ident = nc.dram_tensor("ident", [P, P], F16, kind="ExternalInput").ap()
    out = nc.dram_tensor("out", [S, D], F32, kind="ExternalOutput").ap()
    with tile.TileContext(nc) as tc:
        _build_mha(tc, out, xqT, xkT, xvT, wq, wk, wv, wo, ident)
    nc.compile()
    _CACHED_NC = nc
    return nc


def _run(in_query, in_key, in_value, W_q, W_k, W_v, W_out, **run_kwargs):
    f = lambda a: np.ascontiguousarray(np.asarray(a), dtype=np.float32)
    in_query, in_key, in_value = f(in_query), f(in_key), f(in_value)
    W_q, W_k, W_v, W_out = f(W_q), f(W_k), f(W_v), f(W_out)
    xqT = [f(in_query[b].T) for b in range(B)]
    xkT = [f(in_key[b].T) for b in range(B)]
    xvT = [f(in_value[b].T) for b in range(B)]
    ident = np.eye(P, dtype=np.float16)
    in_maps = []
    for c in range(NCORES):
        b, g = c // 2, c % 2
        in_maps.append(
            {
                "xqT": xqT[b],
                "xkT": xkT[b],
                "xvT": xvT[b],
                "wq": np.ascontiguousarray(W_q[:, g * DG : (g + 1) * DG]),
                "wk": np.ascontiguousarray(W_k[:, g * DG : (g + 1) * DG]),
                "wv": np.ascontiguousarray(W_v[:, g * DG : (g + 1) * DG]),
                "wo": np.ascontiguousarray(
                    W_out[g * DG : (g + 1) * DG, :].astype(np.float16)
                ),
                "ident": ident,
            }
        )
    res = run_bass_kernel_spmd(_get_nc(), in_maps, list(range(NCORES)), **run_kwargs)
    out = np.empty((B, S, D), np.float32)
    for b in range(B):
        out[b] = res.results[2 * b]["out"] + res.results[2 * b + 1]["out"]
    return out, res


def kernel(in_query, in_key, in_value, W_q, W_k, W_v, W_out):
    out, _ = _run(in_query, in_key, in_value, W_q, W_k, W_v, W_out)
    return out
